# revision 15
# baseline (speedup 1.0000x reference)
"""Dynamic Influence Model kernel: builder + host glue.

Device strategy (per core, 8 cores data-parallel over batch B=64):
  - Host gathers + L2-normalizes the neighbor sequences (f32, exact) and
    ships a per-core sequence tensor xT [128(d), R, T, M=512] bf16. The
    device streams it in (r, t-chunk) pieces via plain DMAs.
  - Device runs the per-relation BiLSTM over T=16 steps, both directions
    in lockstep: PE matmuls (bf16, psum f32) per (gate, dir), gate
    activations on ACT with fused per-partition bias, cell/h updates on
    DVE in bf16. tanh(c) is computed once per step as a dir-paired
    [128, 1024] ACT op. The dead f-gate at t=0 (c=0) is skipped.
  - sum_nb relu(h_final) per (relation, direction) -> [128, 8] f32 out.
Host: final small FC chain in float64 (exactly equivalent algebra: the
neighbor-sum is hoisted through the linear layers).
"""
import numpy as np
import ml_dtypes
from dataclasses import dataclass

import concourse.bass as bass
from concourse import mybir, bacc
from concourse.tile import TileContext

F32 = mybir.dt.float32
BF16 = mybir.dt.bfloat16
AF = mybir.ActivationFunctionType
OP = mybir.AluOpType


@dataclass
class Cfg:
    R: int = 3
    T: int = 16
    D: int = 128
    M: int = 512          # sequences per core (= 8 batch * 64 nb)
    NBG: int = 8          # neighbor groups per core (M / 64)
    TCH: int = 4          # timesteps per input DMA chunk


def build_nc(cfg: Cfg):
    R, T, D, M = cfg.R, cfg.T, cfg.D, cfg.M
    H = D

    nc = bacc.Bacc("TRN2", target_bir_lowering=False, num_devices=8)
    xt = nc.dram_tensor("xt", [128, R, T, M], BF16, kind="ExternalInput")
    wih = nc.dram_tensor("wih", [128, R, 2, 4 * H], BF16, kind="ExternalInput")
    whh = nc.dram_tensor("whh", [128, R, 2, 4 * H], BF16, kind="ExternalInput")
    bias = nc.dram_tensor("bias", [128, R, 2, 4], F32, kind="ExternalInput")
    sout = nc.dram_tensor("sout", [R, 2, 128, cfg.NBG], F32, kind="ExternalOutput")

    NCH = T // cfg.TCH    # chunks per relation

    with TileContext(nc) as tc:
        with tc.tile_pool(name="const", bufs=1) as cp, \
             tc.tile_pool(name="xp", bufs=1) as xp, \
             tc.tile_pool(name="gt", bufs=2) as gtp, \
             tc.tile_pool(name="st", bufs=2) as st, \
             tc.tile_pool(name="ot", bufs=1) as otp, \
             tc.tile_pool(name="ps", bufs=1, space="PSUM") as psp:

            # hoist the sigmoid-set ACT_TABLE_LOAD off the critical path: a
            # dummy activation on a dep-free memset tile issues immediately.
            warm = cp.tile([128, 8], F32, name="warm")
            nc.vector.memset(warm[:], 0)
            warm2 = cp.tile([128, 8], BF16, name="warm2")
            nc.scalar.activation(warm2[:], warm[:], AF.Sigmoid)

            wih_t = cp.tile([128, R, 2, 4 * H], BF16)
            bias_t = cp.tile([128, R, 2, 4], F32)
            nc.sync.dma_start(out=wih_t[:, 0], in_=wih[:, 0])
            nc.sync.dma_start(out=bias_t[:], in_=bias[:])
            nc.sync.dma_start(out=wih_t[:, 1:], in_=wih[:, 1:])

            # stream x in per-(relation, t-chunk) DMAs; fwd reads chunks in
            # order, bwd from the top, so load 0, NCH-1, 1, NCH-2, ...
            # whh is not needed until step t=1, so it loads after the first
            # two chunks of r0.
            xr = cp.tile([128, R, T, M], BF16, name="xr")
            order = []
            lo, hi = 0, NCH - 1
            while lo <= hi:
                order.append(lo)
                if hi != lo:
                    order.append(hi)
                lo += 1
                hi -= 1

            # x chunks alternate between the gpsimd and sync DMA rings so
            # the startup chunks land in parallel with the weight loads. The
            # first chunks are split finer so step 0 can start sooner.
            rings = [nc.gpsimd, nc.sync]
            ring_i = [0]

            def load_span(r, t0, t1):
                eng = rings[ring_i[0] % 2]
                ring_i[0] += 1
                eng.dma_start(out=xr[:, r, t0:t1, :], in_=xt[:, r, t0:t1, :])

            load_span(0, 0, 1)       # gpsimd (sync ring is loading wih/bias)
            load_span(0, T - 1, T)   # sync
            whh_t = cp.tile([128, R, 2, 4 * H], BF16)
            nc.sync.dma_start(out=whh_t[:, 0], in_=whh[:, 0])
            load_span(0, 1, 3)
            load_span(0, T - 3, T - 1)
            nc.sync.dma_start(out=whh_t[:, 1:], in_=whh[:, 1:])
            load_span(0, 3, 5)
            load_span(0, T - 5, T - 3)
            load_span(0, 5, 8)
            load_span(0, 8, T - 5)
            for r in range(1, R):
                for ci in order:
                    load_span(r, ci * cfg.TCH, (ci + 1) * cfg.TCH)

            def lstm_step(r, d, te, h, c, first=False):
                gd = {}
                for q in (0, 2, 1, 3):  # i, g first: u1 = sig(i)*tanh(g) early
                    if first and q == 1:
                        continue  # f-gate unused at t=0 (c=0)
                    pq = psp.tile([128, M], F32, tag=f"ps{d}{q}", name="ps")
                    nc.tensor.matmul(pq[:], lhsT=wih_t[:, r, d, q * H:(q + 1) * H],
                                     rhs=xr[:, r, te, :], start=True, stop=first)
                    if not first:
                        nc.tensor.matmul(pq[:], lhsT=whh_t[:, r, d, q * H:(q + 1) * H],
                                         rhs=h[d][:], start=False, stop=True)
                    gq = gtp.tile([128, M], BF16, tag=f"g{d}{q}", name="gq")
                    nc.scalar.activation(gq[:], pq[:],
                                         AF.Tanh if q == 2 else AF.Sigmoid,
                                         bias=bias_t[:, r, d, q:q + 1])
                    gd[q] = gq
                u1 = gtp.tile([128, M], BF16, tag=f"u1{d}", name="u1")
                nc.vector.tensor_tensor(out=u1[:], in0=gd[0][:], in1=gd[2][:],
                                        op=OP.mult)
                if first:
                    c[d] = u1  # c(0) = sig(i)*tanh(g); h,c start at zero
                else:
                    u2 = gtp.tile([128, M], BF16, tag=f"u2{d}", name="u2")
                    nc.vector.tensor_tensor(out=u2[:], in0=gd[1][:], in1=c[d][:],
                                            op=OP.mult)
                    c[d] = st.tile([128, M], BF16, tag=f"c{d}", name="cn")
                    nc.vector.tensor_tensor(out=c[d][:], in0=u1[:], in1=u2[:],
                                            op=OP.add)
                th = gtp.tile([128, M], BF16, tag=f"th{d}", name="th")
                nc.scalar.activation(th[:], c[d][:], AF.Tanh)
                h[d] = st.tile([128, M], BF16, tag=f"h{d}", name="hn")
                nc.vector.tensor_tensor(out=h[d][:], in0=gd[3][:], in1=th[:],
                                        op=OP.mult)

            for r in range(R):
                h = {}
                c = {}
                for t in range(T):
                    lstm_step(r, 0, t, h, c, first=(t == 0))
                    lstm_step(r, 1, T - 1 - t, h, c, first=(t == 0))

                for d in range(2):
                    rl = gtp.tile([128, M], BF16, tag=f"rl{d}", name="rl")
                    nc.vector.tensor_scalar(out=rl[:], in0=h[d][:], scalar1=0.0,
                                            scalar2=None, op0=OP.max)
                    sv = otp.tile([128, cfg.NBG], F32, tag=f"S{d}", name="sv")
                    nc.vector.tensor_reduce(
                        out=sv[:], in_=rl[:].rearrange("p (b n) -> p b n", n=64),
                        op=OP.add, axis=mybir.AxisListType.X)
                    nc.sync.dma_start(out=sout[r, d], in_=sv[:])

    nc.compile()
    return nc


# ---------------- host side ----------------

def prep_x(cfg: Cfg, embeddings, alignment_list, neighbors):
    """Gather + L2-normalize neighbor sequences on host (f32, exact).

    Returns per-core xT arrays [128, R, T, M] bf16 with m = (b_local, nb).
    """
    emb = np.asarray(embeddings)          # [T, N, D] f32
    al = np.asarray(alignment_list)       # [NALIGN, T]
    nbr = np.asarray(neighbors)           # [B, R, NB]
    B, R, NB = nbr.shape
    T, D = cfg.T, cfg.D
    node = al[nbr]                        # [B, R, NB, T]
    x = np.empty((T, B, R, NB, D), dtype=np.float32)
    for t in range(T):
        x[t] = emb[t][node[:, :, :, t]]
    # L2 norm over NB per (t, b, r, d)
    nrm = np.sqrt(np.sum(np.square(x), axis=3, keepdims=True))
    x /= np.maximum(nrm, 1e-12)
    # -> [d, b, r, t, nb] -> per-core [128, R, T, 512]
    xt = np.ascontiguousarray(x.transpose(4, 1, 2, 0, 3)).astype(ml_dtypes.bfloat16)
    cores = []
    for core in range(8):
        blk = xt[:, core * 8:(core + 1) * 8]          # [128, 8, R, T, NB]
        blk = blk.transpose(0, 2, 3, 1, 4).reshape(128, cfg.R, T, cfg.M)
        cores.append(np.ascontiguousarray(blk))
    return cores


def prep_weights(cfg: Cfg, ins):
    H = cfg.D
    wih = np.zeros((128, cfg.R, 2, 4 * H), dtype=ml_dtypes.bfloat16)
    whh = np.zeros((128, cfg.R, 2, 4 * H), dtype=ml_dtypes.bfloat16)
    bias = np.zeros((128, cfg.R, 2, 4), dtype=np.float32)
    for r in range(cfg.R):
        for dirn, sfx in ((0, "_f"), (1, "_b")):
            wih[:, r, dirn, :] = np.asarray(ins["Wih" + sfx][r]).T.astype(ml_dtypes.bfloat16)
            whh[:, r, dirn, :] = np.asarray(ins["Whh" + sfx][r]).T.astype(ml_dtypes.bfloat16)
            b = (np.asarray(ins["bih" + sfx][r]) + np.asarray(ins["bhh" + sfx][r])).astype(np.float32)
            bias[:, r, dirn, :] = b.reshape(4, H).T
    return wih, whh, bias


def finalize(cfg: Cfg, s_cores, ins, nb_total):
    """s_cores: list of [R, 2, 128, NBG] per core -> output [B, OUT] f32."""
    fc_W = np.asarray(ins["fc_W"], np.float64)
    fc_b = np.asarray(ins["fc_b"], np.float64)
    Wsum = np.asarray(ins["W1"], np.float64) + np.asarray(ins["W2"], np.float64)
    Wrel = np.asarray(ins["Wrel"], np.float64)
    outs = []
    for s in s_cores:
        tot = None
        for r in range(cfg.R):
            s_cat = np.concatenate([s[r, 1], s[r, 0]], axis=0).astype(np.float64)  # [2H, NBG]
            o = fc_W[r] @ s_cat + nb_total * fc_b[r][:, None]                      # [OUT, NBG]
            inf = Wrel[r].T @ (Wsum[r].T @ o)                                      # [INF, NBG]
            tot = inf if tot is None else tot + inf
        outs.append(tot.T)  # [NBG, INF] -> batch-local rows
    return np.concatenate(outs, axis=0).astype(np.float32)


# ---------------- self-contained entry point ----------------

_CACHE = {}


def kernel(**inputs):
    """Full-inputs -> full-output Trainium kernel for the Dynamic Influence
    Model. Shards the batch (B=64) over 8 NeuronCores; the host gathers and
    L2-normalizes each core's neighbor sequences, the device runs the
    per-relation BiLSTMs and returns sum_nb relu(h); the tiny trailing FC
    chain is applied on the host in float64 (exactly equivalent algebra -
    the neighbor sum commutes with the linear layers).
    """
    from concourse.bass_utils import run_bass_kernel_spmd

    cfg = _CACHE.get("cfg")
    if cfg is None:
        cfg = Cfg()
        _CACHE["cfg"] = cfg
    nc = _CACHE.get("nc")
    if nc is None:
        nc = build_nc(cfg)
        _CACHE["nc"] = nc

    xt_cores = prep_x(cfg, inputs["embeddings"], inputs["alignment_list"],
                      inputs["neighbors"])
    wih, whh, bias = prep_weights(cfg, inputs)
    in_maps = [{"xt": xt_cores[core], "wih": wih, "whh": whh, "bias": bias}
               for core in range(8)]

    res = run_bass_kernel_spmd(nc, in_maps, list(range(8)))
    s_cores = [res.results[i]["sout"] for i in range(8)]
    return finalize(cfg, s_cores, inputs, nb_total=64)


# revision 17
# speedup vs baseline: 1.0052x; 1.0052x over previous
"""Dynamic Influence Model kernel: builder + host glue.

Device strategy (per core, 8 cores data-parallel over batch B=64):
  - Host gathers + L2-normalizes the neighbor sequences (f32, exact) and
    ships a per-core sequence tensor xT [128(d), R, T, M=512] bf16. The
    device streams it in (r, t-chunk) pieces via plain DMAs.
  - Device runs the per-relation BiLSTM over T=16 steps, both directions
    in lockstep: PE matmuls (bf16, psum f32) per (gate, dir), gate
    activations on ACT with fused per-partition bias, cell/h updates on
    DVE in bf16. tanh(c) is computed once per step as a dir-paired
    [128, 1024] ACT op. The dead f-gate at t=0 (c=0) is skipped.
  - sum_nb relu(h_final) per (relation, direction) -> [128, 8] f32 out.
Host: final small FC chain in float64 (exactly equivalent algebra: the
neighbor-sum is hoisted through the linear layers).
"""
import numpy as np
import ml_dtypes
from dataclasses import dataclass

import concourse.bass as bass
from concourse import mybir, bacc
from concourse.tile import TileContext

F32 = mybir.dt.float32
BF16 = mybir.dt.bfloat16
AF = mybir.ActivationFunctionType
OP = mybir.AluOpType


@dataclass
class Cfg:
    R: int = 3
    T: int = 16
    D: int = 128
    M: int = 512          # sequences per core (= 8 batch * 64 nb)
    NBG: int = 8          # neighbor groups per core (M / 64)
    TCH: int = 4          # timesteps per input DMA chunk


def build_nc(cfg: Cfg):
    R, T, D, M = cfg.R, cfg.T, cfg.D, cfg.M
    H = D

    nc = bacc.Bacc("TRN2", target_bir_lowering=False, num_devices=8)
    xt = nc.dram_tensor("xt", [128, R, T, M], BF16, kind="ExternalInput")
    wih = nc.dram_tensor("wih", [128, R, 2, 4 * H], BF16, kind="ExternalInput")
    whh = nc.dram_tensor("whh", [128, R, 2, 4 * H], BF16, kind="ExternalInput")
    bias = nc.dram_tensor("bias", [128, R, 2, 4], F32, kind="ExternalInput")
    sout = nc.dram_tensor("sout", [R, 2, 128, cfg.NBG], F32, kind="ExternalOutput")

    NCH = T // cfg.TCH    # chunks per relation

    with TileContext(nc) as tc:
        with tc.tile_pool(name="const", bufs=1) as cp, \
             tc.tile_pool(name="xp", bufs=1) as xp, \
             tc.tile_pool(name="gt", bufs=2) as gtp, \
             tc.tile_pool(name="st", bufs=2) as st, \
             tc.tile_pool(name="ot", bufs=1) as otp, \
             tc.tile_pool(name="ps", bufs=1, space="PSUM") as psp:

            # hoist the sigmoid-set ACT_TABLE_LOAD off the critical path: a
            # dummy activation on a dep-free memset tile issues immediately.
            warm = cp.tile([128, 8], F32, name="warm")
            nc.gpsimd.memset(warm[:], 0)
            warm2 = cp.tile([128, 8], BF16, name="warm2")
            nc.scalar.activation(warm2[:], warm[:], AF.Sigmoid)

            wih_t = cp.tile([128, R, 2, 4 * H], BF16)
            bias_t = cp.tile([128, R, 2, 4], F32)
            nc.sync.dma_start(out=wih_t[:, 0], in_=wih[:, 0])
            nc.sync.dma_start(out=bias_t[:], in_=bias[:])
            nc.sync.dma_start(out=wih_t[:, 1:], in_=wih[:, 1:])

            # stream x in per-(relation, t-chunk) DMAs; fwd reads chunks in
            # order, bwd from the top, so load 0, NCH-1, 1, NCH-2, ...
            # whh is not needed until step t=1, so it loads after the first
            # two chunks of r0.
            xr = cp.tile([128, R, T, M], BF16, name="xr")
            order = []
            lo, hi = 0, NCH - 1
            while lo <= hi:
                order.append(lo)
                if hi != lo:
                    order.append(hi)
                lo += 1
                hi -= 1

            # x chunks ride the gpsimd DMA ring, parallel with the weight
            # loads on the sync ring. The first chunks are split finer so
            # step 0 can start sooner.
            def load_span(r, t0, t1):
                nc.gpsimd.dma_start(out=xr[:, r, t0:t1, :],
                                    in_=xt[:, r, t0:t1, :])

            load_span(0, 0, 1)
            load_span(0, T - 1, T)
            whh_t = cp.tile([128, R, 2, 4 * H], BF16)
            nc.sync.dma_start(out=whh_t[:, 0], in_=whh[:, 0])
            load_span(0, 1, 3)
            load_span(0, T - 3, T - 1)
            nc.sync.dma_start(out=whh_t[:, 1:], in_=whh[:, 1:])
            load_span(0, 3, 5)
            load_span(0, T - 5, T - 3)
            load_span(0, 5, 8)
            load_span(0, 8, T - 5)
            for r in range(1, R):
                for ci in order:
                    load_span(r, ci * cfg.TCH, (ci + 1) * cfg.TCH)

            def lstm_step(r, d, te, h, c, first=False):
                gd = {}
                for q in (0, 2, 1, 3):  # i, g first: u1 = sig(i)*tanh(g) early
                    if first and q == 1:
                        continue  # f-gate unused at t=0 (c=0)
                    pq = psp.tile([128, M], F32, tag=f"ps{d}{q}", name="ps")
                    nc.tensor.matmul(pq[:], lhsT=wih_t[:, r, d, q * H:(q + 1) * H],
                                     rhs=xr[:, r, te, :], start=True, stop=first)
                    if not first:
                        nc.tensor.matmul(pq[:], lhsT=whh_t[:, r, d, q * H:(q + 1) * H],
                                         rhs=h[d][:], start=False, stop=True)
                    gq = gtp.tile([128, M], BF16, tag=f"g{d}{q}", name="gq")
                    nc.scalar.activation(gq[:], pq[:],
                                         AF.Tanh if q == 2 else AF.Sigmoid,
                                         bias=bias_t[:, r, d, q:q + 1])
                    gd[q] = gq
                u1 = gtp.tile([128, M], BF16, tag=f"u1{d}", name="u1")
                nc.vector.tensor_tensor(out=u1[:], in0=gd[0][:], in1=gd[2][:],
                                        op=OP.mult)
                if first:
                    c[d] = u1  # c(0) = sig(i)*tanh(g); h,c start at zero
                else:
                    u2 = gtp.tile([128, M], BF16, tag=f"u2{d}", name="u2")
                    nc.vector.tensor_tensor(out=u2[:], in0=gd[1][:], in1=c[d][:],
                                            op=OP.mult)
                    c[d] = st.tile([128, M], BF16, tag=f"c{d}", name="cn")
                    nc.vector.tensor_tensor(out=c[d][:], in0=u1[:], in1=u2[:],
                                            op=OP.add)
                th = gtp.tile([128, M], BF16, tag=f"th{d}", name="th")
                nc.scalar.activation(th[:], c[d][:], AF.Tanh)
                h[d] = st.tile([128, M], BF16, tag=f"h{d}", name="hn")
                nc.vector.tensor_tensor(out=h[d][:], in0=gd[3][:], in1=th[:],
                                        op=OP.mult)

            for r in range(R):
                h = {}
                c = {}
                for t in range(T):
                    lstm_step(r, 0, t, h, c, first=(t == 0))
                    lstm_step(r, 1, T - 1 - t, h, c, first=(t == 0))

                for d in range(2):
                    rl = gtp.tile([128, M], BF16, tag=f"rl{d}", name="rl")
                    nc.vector.tensor_scalar(out=rl[:], in0=h[d][:], scalar1=0.0,
                                            scalar2=None, op0=OP.max)
                    sv = otp.tile([128, cfg.NBG], F32, tag=f"S{d}", name="sv")
                    nc.vector.tensor_reduce(
                        out=sv[:], in_=rl[:].rearrange("p (b n) -> p b n", n=64),
                        op=OP.add, axis=mybir.AxisListType.X)
                    nc.sync.dma_start(out=sout[r, d], in_=sv[:])

    nc.compile()
    return nc


# ---------------- host side ----------------

def prep_x(cfg: Cfg, embeddings, alignment_list, neighbors):
    """Gather + L2-normalize neighbor sequences on host (f32, exact).

    Returns per-core xT arrays [128, R, T, M] bf16 with m = (b_local, nb).
    """
    emb = np.asarray(embeddings)          # [T, N, D] f32
    al = np.asarray(alignment_list)       # [NALIGN, T]
    nbr = np.asarray(neighbors)           # [B, R, NB]
    B, R, NB = nbr.shape
    T, D = cfg.T, cfg.D
    node = al[nbr]                        # [B, R, NB, T]
    x = np.empty((T, B, R, NB, D), dtype=np.float32)
    for t in range(T):
        x[t] = emb[t][node[:, :, :, t]]
    # L2 norm over NB per (t, b, r, d)
    nrm = np.sqrt(np.sum(np.square(x), axis=3, keepdims=True))
    x /= np.maximum(nrm, 1e-12)
    # -> [d, b, r, t, nb] -> per-core [128, R, T, 512]
    xt = np.ascontiguousarray(x.transpose(4, 1, 2, 0, 3)).astype(ml_dtypes.bfloat16)
    cores = []
    for core in range(8):
        blk = xt[:, core * 8:(core + 1) * 8]          # [128, 8, R, T, NB]
        blk = blk.transpose(0, 2, 3, 1, 4).reshape(128, cfg.R, T, cfg.M)
        cores.append(np.ascontiguousarray(blk))
    return cores


def prep_weights(cfg: Cfg, ins):
    H = cfg.D
    wih = np.zeros((128, cfg.R, 2, 4 * H), dtype=ml_dtypes.bfloat16)
    whh = np.zeros((128, cfg.R, 2, 4 * H), dtype=ml_dtypes.bfloat16)
    bias = np.zeros((128, cfg.R, 2, 4), dtype=np.float32)
    for r in range(cfg.R):
        for dirn, sfx in ((0, "_f"), (1, "_b")):
            wih[:, r, dirn, :] = np.asarray(ins["Wih" + sfx][r]).T.astype(ml_dtypes.bfloat16)
            whh[:, r, dirn, :] = np.asarray(ins["Whh" + sfx][r]).T.astype(ml_dtypes.bfloat16)
            b = (np.asarray(ins["bih" + sfx][r]) + np.asarray(ins["bhh" + sfx][r])).astype(np.float32)
            bias[:, r, dirn, :] = b.reshape(4, H).T
    return wih, whh, bias


def finalize(cfg: Cfg, s_cores, ins, nb_total):
    """s_cores: list of [R, 2, 128, NBG] per core -> output [B, OUT] f32."""
    fc_W = np.asarray(ins["fc_W"], np.float64)
    fc_b = np.asarray(ins["fc_b"], np.float64)
    Wsum = np.asarray(ins["W1"], np.float64) + np.asarray(ins["W2"], np.float64)
    Wrel = np.asarray(ins["Wrel"], np.float64)
    outs = []
    for s in s_cores:
        tot = None
        for r in range(cfg.R):
            s_cat = np.concatenate([s[r, 1], s[r, 0]], axis=0).astype(np.float64)  # [2H, NBG]
            o = fc_W[r] @ s_cat + nb_total * fc_b[r][:, None]                      # [OUT, NBG]
            inf = Wrel[r].T @ (Wsum[r].T @ o)                                      # [INF, NBG]
            tot = inf if tot is None else tot + inf
        outs.append(tot.T)  # [NBG, INF] -> batch-local rows
    return np.concatenate(outs, axis=0).astype(np.float32)


# ---------------- self-contained entry point ----------------

_CACHE = {}


def kernel(**inputs):
    """Full-inputs -> full-output Trainium kernel for the Dynamic Influence
    Model. Shards the batch (B=64) over 8 NeuronCores; the host gathers and
    L2-normalizes each core's neighbor sequences, the device runs the
    per-relation BiLSTMs and returns sum_nb relu(h); the tiny trailing FC
    chain is applied on the host in float64 (exactly equivalent algebra -
    the neighbor sum commutes with the linear layers).
    """
    from concourse.bass_utils import run_bass_kernel_spmd

    cfg = _CACHE.get("cfg")
    if cfg is None:
        cfg = Cfg()
        _CACHE["cfg"] = cfg
    nc = _CACHE.get("nc")
    if nc is None:
        nc = build_nc(cfg)
        _CACHE["nc"] = nc

    xt_cores = prep_x(cfg, inputs["embeddings"], inputs["alignment_list"],
                      inputs["neighbors"])
    wih, whh, bias = prep_weights(cfg, inputs)
    in_maps = [{"xt": xt_cores[core], "wih": wih, "whh": whh, "bias": bias}
               for core in range(8)]

    res = run_bass_kernel_spmd(nc, in_maps, list(range(8)))
    s_cores = [res.results[i]["sout"] for i in range(8)]
    return finalize(cfg, s_cores, inputs, nb_total=64)


# revision 19
# speedup vs baseline: 1.0095x; 1.0043x over previous
"""Dynamic Influence Model kernel: builder + host glue.

Device strategy (per core, 8 cores data-parallel over batch B=64):
  - Host gathers + L2-normalizes the neighbor sequences (f32, exact) and
    ships a per-core sequence tensor xT [128(d), R, T, M=512] bf16. The
    device streams it in (r, t-chunk) pieces via plain DMAs.
  - Device runs the per-relation BiLSTM over T=16 steps, both directions
    in lockstep: PE matmuls (bf16, psum f32) per (gate, dir), gate
    activations on ACT with fused per-partition bias, cell/h updates on
    DVE in bf16. tanh(c) is computed once per step as a dir-paired
    [128, 1024] ACT op. The dead f-gate at t=0 (c=0) is skipped.
  - sum_nb relu(h_final) per (relation, direction) -> [128, 8] f32 out.
Host: final small FC chain in float64 (exactly equivalent algebra: the
neighbor-sum is hoisted through the linear layers).
"""
import numpy as np
import ml_dtypes
from dataclasses import dataclass

import concourse.bass as bass
from concourse import mybir, bacc
from concourse.tile import TileContext

F32 = mybir.dt.float32
BF16 = mybir.dt.bfloat16
AF = mybir.ActivationFunctionType
OP = mybir.AluOpType


@dataclass
class Cfg:
    R: int = 3
    T: int = 16
    D: int = 128
    M: int = 512          # sequences per core (= 8 batch * 64 nb)
    NBG: int = 8          # neighbor groups per core (M / 64)
    TCH: int = 4          # timesteps per input DMA chunk


def build_nc(cfg: Cfg):
    R, T, D, M = cfg.R, cfg.T, cfg.D, cfg.M
    H = D

    nc = bacc.Bacc("TRN2", target_bir_lowering=False, num_devices=8)
    xt = nc.dram_tensor("xt", [128, R, T, M], BF16, kind="ExternalInput")
    wih = nc.dram_tensor("wih", [128, R, 2, 4 * H], BF16, kind="ExternalInput")
    whh = nc.dram_tensor("whh", [128, R, 2, 4 * H], BF16, kind="ExternalInput")
    bias = nc.dram_tensor("bias", [128, R, 2, 4], F32, kind="ExternalInput")
    sout = nc.dram_tensor("sout", [R, 2, 128, cfg.NBG], F32, kind="ExternalOutput")

    NCH = T // cfg.TCH    # chunks per relation

    with TileContext(nc) as tc:
        with tc.tile_pool(name="const", bufs=1) as cp, \
             tc.tile_pool(name="xp", bufs=1) as xp, \
             tc.tile_pool(name="gt", bufs=2) as gtp, \
             tc.tile_pool(name="st", bufs=2) as st, \
             tc.tile_pool(name="ot", bufs=1) as otp, \
             tc.tile_pool(name="ps", bufs=1, space="PSUM") as psp:

            # hoist the sigmoid-set ACT_TABLE_LOAD off the critical path: a
            # dummy activation on a dep-free memset tile issues immediately.
            warm = cp.tile([128, 8], F32, name="warm")
            nc.gpsimd.memset(warm[:], 0)
            warm2 = cp.tile([128, 8], BF16, name="warm2")
            nc.scalar.activation(warm2[:], warm[:], AF.Sigmoid)

            wih_t = cp.tile([128, R, 2, 4 * H], BF16)
            bias_t = cp.tile([128, R, 2, 4], F32)
            nc.sync.dma_start(out=wih_t[:, 0], in_=wih[:, 0])
            nc.sync.dma_start(out=bias_t[:], in_=bias[:])
            nc.sync.dma_start(out=wih_t[:, 1:], in_=wih[:, 1:])

            # stream x in per-(relation, t-chunk) DMAs; fwd reads chunks in
            # order, bwd from the top, so load 0, NCH-1, 1, NCH-2, ...
            # whh is not needed until step t=1, so it loads after the first
            # two chunks of r0.
            xr = cp.tile([128, R, T, M], BF16, name="xr")
            order = []
            lo, hi = 0, NCH - 1
            while lo <= hi:
                order.append(lo)
                if hi != lo:
                    order.append(hi)
                lo += 1
                hi -= 1

            # Early-needed x chunks ride the gpsimd DMA ring, parallel with
            # the weight loads on the sync ring; late chunks follow on the
            # sync ring behind the weights. The first chunks are split finer
            # so step 0 can start sooner.
            def load_span(r, t0, t1, eng=None):
                (eng or nc.gpsimd).dma_start(out=xr[:, r, t0:t1, :],
                                             in_=xt[:, r, t0:t1, :])

            load_span(0, 0, 1)
            load_span(0, T - 1, T)
            whh_t = cp.tile([128, R, 2, 4 * H], BF16)
            nc.sync.dma_start(out=whh_t[:, 0], in_=whh[:, 0])
            load_span(0, 1, 3)
            load_span(0, T - 3, T - 1)
            nc.sync.dma_start(out=whh_t[:, 1:], in_=whh[:, 1:])
            load_span(0, 3, 5)
            load_span(0, T - 5, T - 3, eng=nc.sync)
            load_span(0, 5, 8)
            load_span(0, 8, T - 5, eng=nc.sync)
            for i, (r, ci) in enumerate([(r, ci) for r in range(1, R)
                                         for ci in order]):
                load_span(r, ci * cfg.TCH, (ci + 1) * cfg.TCH,
                          eng=nc.sync if i % 2 else nc.gpsimd)

            def lstm_step(r, d, te, h, c, first=False):
                gd = {}
                for q in (0, 2, 1, 3):  # i, g first: u1 = sig(i)*tanh(g) early
                    if first and q == 1:
                        continue  # f-gate unused at t=0 (c=0)
                    pq = psp.tile([128, M], F32, tag=f"ps{d}{q}", name="ps")
                    nc.tensor.matmul(pq[:], lhsT=wih_t[:, r, d, q * H:(q + 1) * H],
                                     rhs=xr[:, r, te, :], start=True, stop=first)
                    if not first:
                        nc.tensor.matmul(pq[:], lhsT=whh_t[:, r, d, q * H:(q + 1) * H],
                                         rhs=h[d][:], start=False, stop=True)
                    gq = gtp.tile([128, M], BF16, tag=f"g{d}{q}", name="gq")
                    nc.scalar.activation(gq[:], pq[:],
                                         AF.Tanh if q == 2 else AF.Sigmoid,
                                         bias=bias_t[:, r, d, q:q + 1])
                    gd[q] = gq
                u1 = gtp.tile([128, M], BF16, tag=f"u1{d}", name="u1")
                nc.vector.tensor_tensor(out=u1[:], in0=gd[0][:], in1=gd[2][:],
                                        op=OP.mult)
                if first:
                    c[d] = u1  # c(0) = sig(i)*tanh(g); h,c start at zero
                else:
                    u2 = gtp.tile([128, M], BF16, tag=f"u2{d}", name="u2")
                    nc.vector.tensor_tensor(out=u2[:], in0=gd[1][:], in1=c[d][:],
                                            op=OP.mult)
                    c[d] = st.tile([128, M], BF16, tag=f"c{d}", name="cn")
                    nc.vector.tensor_tensor(out=c[d][:], in0=u1[:], in1=u2[:],
                                            op=OP.add)
                th = gtp.tile([128, M], BF16, tag=f"th{d}", name="th")
                nc.scalar.activation(th[:], c[d][:], AF.Tanh)
                h[d] = st.tile([128, M], BF16, tag=f"h{d}", name="hn")
                nc.vector.tensor_tensor(out=h[d][:], in0=gd[3][:], in1=th[:],
                                        op=OP.mult)

            for r in range(R):
                h = {}
                c = {}
                for t in range(T):
                    lstm_step(r, 0, t, h, c, first=(t == 0))
                    lstm_step(r, 1, T - 1 - t, h, c, first=(t == 0))

                for d in range(2):
                    rl = gtp.tile([128, M], BF16, tag=f"rl{d}", name="rl")
                    nc.vector.tensor_scalar(out=rl[:], in0=h[d][:], scalar1=0.0,
                                            scalar2=None, op0=OP.max)
                    sv = otp.tile([128, cfg.NBG], F32, tag=f"S{d}", name="sv")
                    nc.vector.tensor_reduce(
                        out=sv[:], in_=rl[:].rearrange("p (b n) -> p b n", n=64),
                        op=OP.add, axis=mybir.AxisListType.X)
                    nc.sync.dma_start(out=sout[r, d], in_=sv[:])

    nc.compile()
    return nc


# ---------------- host side ----------------

def prep_x(cfg: Cfg, embeddings, alignment_list, neighbors):
    """Gather + L2-normalize neighbor sequences on host (f32, exact).

    Returns per-core xT arrays [128, R, T, M] bf16 with m = (b_local, nb).
    """
    emb = np.asarray(embeddings)          # [T, N, D] f32
    al = np.asarray(alignment_list)       # [NALIGN, T]
    nbr = np.asarray(neighbors)           # [B, R, NB]
    B, R, NB = nbr.shape
    T, D = cfg.T, cfg.D
    node = al[nbr]                        # [B, R, NB, T]
    x = np.empty((T, B, R, NB, D), dtype=np.float32)
    for t in range(T):
        x[t] = emb[t][node[:, :, :, t]]
    # L2 norm over NB per (t, b, r, d)
    nrm = np.sqrt(np.sum(np.square(x), axis=3, keepdims=True))
    x /= np.maximum(nrm, 1e-12)
    # -> [d, b, r, t, nb] -> per-core [128, R, T, 512]
    xt = np.ascontiguousarray(x.transpose(4, 1, 2, 0, 3)).astype(ml_dtypes.bfloat16)
    cores = []
    for core in range(8):
        blk = xt[:, core * 8:(core + 1) * 8]          # [128, 8, R, T, NB]
        blk = blk.transpose(0, 2, 3, 1, 4).reshape(128, cfg.R, T, cfg.M)
        cores.append(np.ascontiguousarray(blk))
    return cores


def prep_weights(cfg: Cfg, ins):
    H = cfg.D
    wih = np.zeros((128, cfg.R, 2, 4 * H), dtype=ml_dtypes.bfloat16)
    whh = np.zeros((128, cfg.R, 2, 4 * H), dtype=ml_dtypes.bfloat16)
    bias = np.zeros((128, cfg.R, 2, 4), dtype=np.float32)
    for r in range(cfg.R):
        for dirn, sfx in ((0, "_f"), (1, "_b")):
            wih[:, r, dirn, :] = np.asarray(ins["Wih" + sfx][r]).T.astype(ml_dtypes.bfloat16)
            whh[:, r, dirn, :] = np.asarray(ins["Whh" + sfx][r]).T.astype(ml_dtypes.bfloat16)
            b = (np.asarray(ins["bih" + sfx][r]) + np.asarray(ins["bhh" + sfx][r])).astype(np.float32)
            bias[:, r, dirn, :] = b.reshape(4, H).T
    return wih, whh, bias


def finalize(cfg: Cfg, s_cores, ins, nb_total):
    """s_cores: list of [R, 2, 128, NBG] per core -> output [B, OUT] f32."""
    fc_W = np.asarray(ins["fc_W"], np.float64)
    fc_b = np.asarray(ins["fc_b"], np.float64)
    Wsum = np.asarray(ins["W1"], np.float64) + np.asarray(ins["W2"], np.float64)
    Wrel = np.asarray(ins["Wrel"], np.float64)
    outs = []
    for s in s_cores:
        tot = None
        for r in range(cfg.R):
            s_cat = np.concatenate([s[r, 1], s[r, 0]], axis=0).astype(np.float64)  # [2H, NBG]
            o = fc_W[r] @ s_cat + nb_total * fc_b[r][:, None]                      # [OUT, NBG]
            inf = Wrel[r].T @ (Wsum[r].T @ o)                                      # [INF, NBG]
            tot = inf if tot is None else tot + inf
        outs.append(tot.T)  # [NBG, INF] -> batch-local rows
    return np.concatenate(outs, axis=0).astype(np.float32)


# ---------------- self-contained entry point ----------------

_CACHE = {}


def kernel(**inputs):
    """Full-inputs -> full-output Trainium kernel for the Dynamic Influence
    Model. Shards the batch (B=64) over 8 NeuronCores; the host gathers and
    L2-normalizes each core's neighbor sequences, the device runs the
    per-relation BiLSTMs and returns sum_nb relu(h); the tiny trailing FC
    chain is applied on the host in float64 (exactly equivalent algebra -
    the neighbor sum commutes with the linear layers).
    """
    from concourse.bass_utils import run_bass_kernel_spmd

    cfg = _CACHE.get("cfg")
    if cfg is None:
        cfg = Cfg()
        _CACHE["cfg"] = cfg
    nc = _CACHE.get("nc")
    if nc is None:
        nc = build_nc(cfg)
        _CACHE["nc"] = nc

    xt_cores = prep_x(cfg, inputs["embeddings"], inputs["alignment_list"],
                      inputs["neighbors"])
    wih, whh, bias = prep_weights(cfg, inputs)
    in_maps = [{"xt": xt_cores[core], "wih": wih, "whh": whh, "bias": bias}
               for core in range(8)]

    res = run_bass_kernel_spmd(nc, in_maps, list(range(8)))
    s_cores = [res.results[i]["sout"] for i in range(8)]
    return finalize(cfg, s_cores, inputs, nb_total=64)
